# revision 6
# baseline (speedup 1.0000x reference)
"""CLIP contrastive loss on 8 Trainium2 NeuronCores.

loss = 0.5 * (mean(lse_rows - diag) + mean(lse_cols - diag)) of
logits = logit_scale * image_features @ text_features.T, N=16384, D=512.

Strategy: row-shard both modalities across 8 cores. Each core runs two
symmetric passes of a fused matmul+online-logsumexp kernel:
  pass A: rows of logits  = (s*img_c) @ txt^T  -> row LSE + diag (local rows)
  pass B: rows of logits^T = (s*txt_c) @ img^T -> col LSE (local cols, exact)
Matmul inputs are bf16 (fp32 PSUM accumulation); online-softmax stats are
fp32. Each core emits per-row running sums (rs) and negated maxes (nm); the
host finishes lse = log(rs) - nm and the scalar loss in fp64 (keeps the Ln
activation-table swap off the ScalarE hot path).
"""

import sys

for _p in ("/opt/trn_rl_repo", "/root/.axon_site/_ro/trn_rl_repo"):
    if _p not in sys.path:
        sys.path.insert(0, _p)

from contextlib import ExitStack

import ml_dtypes
import numpy as np

import concourse.bass as bass
import concourse.mybir as mybir
import concourse.tile as tile
from concourse import bacc
from concourse.bass_utils import run_bass_kernel_spmd

F32 = mybir.dt.float32
BF16 = mybir.dt.bfloat16
AX = mybir.AxisListType.X
ALU = mybir.AluOpType
ACTF = mybir.ActivationFunctionType

N_TOTAL = 16384
D = 512
N_CORES = 8
P = 128
CHUNK = 1024  # logit columns per PSUM tile (2 banks)
MM_N = 512    # matmul moving free dim (fp32 PSUM output: one bank max)


def build_clip_nc(n_total=N_TOTAL, d=D, n_cores=N_CORES, chunk=CHUNK,
                  mm_n=MM_N, repeat=1):
    m_loc = n_total // n_cores      # local rows per core
    m_tiles = m_loc // P            # partition tiles of local rows
    kch = d // P                    # contraction chunks
    n_chunks = n_total // chunk     # column chunks
    n_sub = chunk // mm_n           # matmuls per column chunk per k

    nc = bacc.Bacc(
        "TRN2", target_bir_lowering=False, debug=False, num_devices=n_cores
    )

    qt_a = nc.dram_tensor("qt_a", [d, m_loc], BF16, kind="ExternalInput")
    kt_a = nc.dram_tensor("kt_a", [d, n_total], BF16, kind="ExternalInput")
    qt_b = nc.dram_tensor("qt_b", [d, m_loc], BF16, kind="ExternalInput")
    kt_b = nc.dram_tensor("kt_b", [d, n_total], BF16, kind="ExternalInput")
    qrow = nc.dram_tensor("qrow", [m_loc, d], BF16, kind="ExternalInput")
    krow = nc.dram_tensor("krow", [m_loc, d], BF16, kind="ExternalInput")

    outs = {}
    for nm_ in ("rs_a", "nm_a", "rs_b", "nm_b", "diag"):
        outs[nm_] = nc.dram_tensor(nm_, [P, m_tiles], F32, kind="ExternalOutput")

    with ExitStack() as ctx:
        tc = ctx.enter_context(tile.TileContext(nc))
        kt_pool = ctx.enter_context(tc.tile_pool(name="kt", bufs=1))
        qt_pool = ctx.enter_context(tc.tile_pool(name="qt", bufs=1))
        row_pool = ctx.enter_context(tc.tile_pool(name="rows", bufs=1))
        stage_pool = ctx.enter_context(tc.tile_pool(name="stage", bufs=1))
        esc_pool = ctx.enter_context(tc.tile_pool(name="esc", bufs=3))
        sm_pool = ctx.enter_context(tc.tile_pool(name="sm", bufs=6))
        psum_pool = ctx.enter_context(
            tc.tile_pool(name="ps", bufs=4, space="PSUM")
        )

        def one_pass(qt_dram, kt_dram, rs_out, nm_out, with_diag):
            # ---- loads ----
            qts = []
            for k in range(kch):
                t = qt_pool.tile([P, m_loc], BF16, tag=f"qt{k}", name=f"qt{k}")
                nc.sync.dma_start(t[:], qt_dram[k * P:(k + 1) * P, :])
                qts.append(t)
            kts = {}
            for pc in range(n_chunks):
                for k in range(kch):
                    t = kt_pool.tile([P, chunk], BF16, tag=f"kt{k}_{pc}",
                                     name=f"kt{k}_{pc}")
                    nc.sync.dma_start(
                        t[:],
                        kt_dram[k * P:(k + 1) * P, pc * chunk:(pc + 1) * chunk],
                    )
                    kts[(k, pc)] = t
            if with_diag:
                qrow_sb = row_pool.tile([P, m_tiles * d], BF16, tag="qrow",
                                        name="qrow_sb")
                krow_sb = row_pool.tile([P, m_tiles * d], BF16, tag="krow",
                                        name="krow_sb")
                for m in range(m_tiles):
                    nc.sync.dma_start(
                        qrow_sb[:, m * d:(m + 1) * d],
                        qrow[m * P:(m + 1) * P, :],
                    )
                    nc.sync.dma_start(
                        krow_sb[:, m * d:(m + 1) * d],
                        krow[m * P:(m + 1) * P, :],
                    )
                diag_stage = stage_pool.tile([P, m_tiles], F32, tag="diag_st",
                                             name="diag_stage")

            sfx = "a" if with_diag else "b"
            rs_stage = stage_pool.tile([P, m_tiles], F32, tag=f"rs_st_{sfx}",
                                       name=f"rs_stage_{sfx}")
            nm_stage = stage_pool.tile([P, m_tiles], F32, tag=f"nm_st_{sfx}",
                                       name=f"nm_stage_{sfx}")

            # ---- main loop ----
            for m in range(m_tiles):
                nm = None  # negated running row max
                rs = None  # running sum of exp(x - max)
                for c in range(n_chunks):
                    ps = psum_pool.tile([P, chunk], F32, tag="ps", name="ps")
                    for k in range(kch):
                        lhsT = qts[k][:, m * P:(m + 1) * P]
                        for s in range(n_sub):
                            nc.tensor.matmul(
                                ps[:, s * mm_n:(s + 1) * mm_n],
                                lhsT,
                                kts[(k, c)][:, s * mm_n:(s + 1) * mm_n],
                                start=(k == 0),
                                stop=(k == kch - 1),
                            )
                    negmax = sm_pool.tile([P, 1], F32, tag="negmax",
                                          name="negmax")
                    nc.vector.reduce_max(negmax[:], ps[:], axis=AX, negate=True)
                    if c == 0:
                        nm_new = negmax
                    else:
                        nm_new = sm_pool.tile([P, 1], F32, tag="nm", name="nm")
                        nc.vector.tensor_tensor(
                            nm_new[:], nm[:], negmax[:], op=ALU.min
                        )
                        corr = sm_pool.tile([P, 1], F32, tag="corr", name="corr")
                        # corr = exp(nm_new - nm_old)
                        nc.scalar.activation(
                            corr[:], nm[:], ACTF.Exp, bias=nm_new[:], scale=-1.0
                        )
                    esc = esc_pool.tile([P, chunk], BF16, tag="esc", name="esc")
                    csum = sm_pool.tile([P, 1], F32, tag="csum", name="csum")
                    # esc = exp(ps + nm_new); csum = row-sum of esc (fp32)
                    nc.scalar.activation(
                        esc[:], ps[:], ACTF.Exp, bias=nm_new[:], scale=1.0,
                        accum_out=csum[:],
                    )
                    if c == 0:
                        rs_new = csum
                    else:
                        tmp = sm_pool.tile([P, 1], F32, tag="tmp", name="tmp")
                        nc.vector.tensor_tensor(tmp[:], rs[:], corr[:],
                                                op=ALU.mult)
                        rs_new = sm_pool.tile([P, 1], F32, tag="rs", name="rs")
                        nc.vector.tensor_tensor(
                            rs_new[:], tmp[:], csum[:], op=ALU.add
                        )
                    nm, rs = nm_new, rs_new
                # stage running stats; host computes lse = log(rs) - nm
                nc.vector.tensor_copy(rs_stage[:, m:m + 1], rs[:])
                nc.vector.tensor_copy(nm_stage[:, m:m + 1], nm[:])
                if with_diag:
                    # diag row-dot: out = (qrow * 1.0) * krow, accum = row sum
                    # (InstTensorTensorReduce crashes TRN2 HW; this
                    # InstTensorScalarPtr form is verified working.)
                    dsc = esc_pool.tile([P, d], F32, tag="dsc", name="dsc")
                    nc.vector.scalar_tensor_tensor(
                        out=dsc[:],
                        in0=qrow_sb[:, m * d:(m + 1) * d],
                        scalar=1.0,
                        in1=krow_sb[:, m * d:(m + 1) * d],
                        op0=ALU.mult,
                        op1=ALU.mult,
                        accum_out=diag_stage[:, m:m + 1],
                    )

            nc.sync.dma_start(rs_out[:], rs_stage[:])
            nc.sync.dma_start(nm_out[:], nm_stage[:])
            if with_diag:
                nc.sync.dma_start(outs["diag"][:], diag_stage[:])

        for _rep in range(repeat):  # >1 only for timing measurements
            one_pass(qt_a, kt_a, outs["rs_a"], outs["nm_a"], True)
            one_pass(qt_b, kt_b, outs["rs_b"], outs["nm_b"], False)

    nc.compile()
    return nc


_NC_CACHE = {}


def _get_nc():
    key = (N_TOTAL, D, N_CORES, CHUNK)
    if key not in _NC_CACHE:
        _NC_CACHE[key] = build_clip_nc(N_TOTAL, D, N_CORES, CHUNK)
    return _NC_CACHE[key]


def make_in_maps(image_features, text_features, logit_scale,
                 n_total=N_TOTAL, n_cores=N_CORES):
    """Host-side sharding: per-core input dicts (all bf16)."""
    bf = ml_dtypes.bfloat16
    img = np.asarray(image_features, np.float32)
    txt = np.asarray(text_features, np.float32)
    s = np.float32(logit_scale)
    m_loc = n_total // n_cores

    img_s = (img * s).astype(bf)          # [N, D] scaled, bf16
    txt_b = txt.astype(bf)                # [N, D]
    txt_s = (txt * s).astype(bf)
    img_b = img.astype(bf)
    kt_a = np.ascontiguousarray(txt_b.T)  # [D, N] shared by all cores
    kt_b = np.ascontiguousarray(img_b.T)

    in_maps = []
    for c in range(n_cores):
        rows = slice(c * m_loc, (c + 1) * m_loc)
        in_maps.append({
            "qt_a": np.ascontiguousarray(img_s[rows].T),
            "kt_a": kt_a,
            "qt_b": np.ascontiguousarray(txt_s[rows].T),
            "kt_b": kt_b,
            "qrow": np.ascontiguousarray(img_s[rows]),
            "krow": np.ascontiguousarray(txt_b[rows]),
        })
    return in_maps


def combine_results(results, n_total=N_TOTAL, n_cores=N_CORES):
    """Gather per-core [P, m_tiles] stats into the scalar loss (fp64)."""
    def gather(name):
        # stage[p, m] holds value for local row m*P + p
        return np.concatenate(
            [np.asarray(r[name], np.float64).T.reshape(-1) for r in results]
        )

    lse_r = np.log(gather("rs_a")) - gather("nm_a")
    lse_c = np.log(gather("rs_b")) - gather("nm_b")
    diag = gather("diag")
    loss = 0.5 * ((lse_r - diag).mean() + (lse_c - diag).mean())
    return np.float32(loss)


def kernel(image_features, text_features, logit_scale):
    nc = _get_nc()
    in_maps = make_in_maps(image_features, text_features, logit_scale)
    res = run_bass_kernel_spmd(nc, in_maps, list(range(N_CORES))).results
    return combine_results(res)


if __name__ == "__main__":
    rng = np.random.default_rng(0)
    out = kernel(
        rng.standard_normal((N_TOTAL, D)).astype(np.float32),
        rng.standard_normal((N_TOTAL, D)).astype(np.float32),
        np.float32(100.0),
    )
    print("loss:", out)
